# revision 29
# baseline (speedup 1.0000x reference)
"""CrossViewAttention Trainium2 kernel (v7).

Shards the B*V=16 (batch, view) attention instances across 8 NeuronCores,
2 per core, paired as (b, v) and (b, v+2) so the two instances share KV
source view v+1 (each instance attends over views v-1, v+1 circular).
Per core the 3 distinct KV source views are projected once (25% fewer
K/V projection FLOPs). All matmul operands are bf16 (fp32 PSUM).

Schedule (single rotating 8-bank PSUM pool, no phase barriers):
  A1  K^T = wk^T @ x_kv^T   [feat, t]   (drain split ACT+DVE -> KT bf16)
  A2  V   = x_kv @ wv       [t, feat]   (+ ones col -> VA, DVE drain)
  A3+B software pipeline per head-pair j (lag 1):
      emit QK(j-1) -> emit A3(j) -> emit PV(j-1)
      so the ACT-engine exp of step j-1 hides under A3(j) matmuls.
  Softmax denominators l come free from PV's ones column. Per group the
  l row is staged at partition 0 (DVE), DMA-scattered into an
  [8-row x batch] quadrant-aligned stack, and each batch of 8 groups
  gets ONE [8,512] DVE reciprocal (HW reciprocal costs free-size*8cyc
  regardless of partition count). The 1/l rows are column-folded back
  to partition 0 by one DMA per batch; normalization (gpsimd
  partition_broadcast + in-place multiplies, all on the otherwise-idle
  Pool engine) is paced 2 groups per pipeline step so it never gates
  phase C. partition_broadcast HW quirk: dst/src partition base must
  be 0 (src column offsets are fine) - hence the fold to partition 0.
  C   y = O @ wo  (nn pairs, 8 psum banks, [128,1024] wo tiles; yt
      drains on DVE, y DMAs spread across queues)
"""
import numpy as np
import ml_dtypes

B, V, S, D = 2, 8, 256, 2048
NH, NKV, KVR = 32, 8, 2
HD = D // NH  # 64
G = NH // NKV  # 4
N_CORES = 8
P = 2  # instances per core
SCALE = 1.0 / np.sqrt(HD)
BFNP = ml_dtypes.bfloat16
V0S = (0, 1, 4, 5)  # per-core first view; pair is (v0, v0+2)

_CACHE = {}


def _build():
    import concourse.tile as tile
    import concourse.mybir as mybir
    from concourse import bacc
    from contextlib import ExitStack

    F32 = mybir.dt.float32
    BF16 = mybir.dt.bfloat16
    Exp = mybir.ActivationFunctionType.Exp
    Ln = mybir.ActivationFunctionType.Ln
    Mult = mybir.AluOpType.mult

    nc = bacc.Bacc("TRN2", target_bir_lowering=False, debug=False,
                   num_devices=N_CORES)
    xqT = nc.dram_tensor("xqT", [D, P * S], BF16, kind="ExternalInput").ap()
    xkvT = nc.dram_tensor("xkvT", [D, 768], BF16, kind="ExternalInput").ap()
    wq = nc.dram_tensor("wq", [D, D], BF16, kind="ExternalInput").ap()
    wkv = nc.dram_tensor("wkv", [D, 1024], BF16, kind="ExternalInput").ap()
    wo = nc.dram_tensor("wo", [D, D], BF16, kind="ExternalInput").ap()
    y = nc.dram_tensor("y", [P * S, D], F32, kind="ExternalOutput").ap()

    with tile.TileContext(nc) as tc, ExitStack() as top:
        xp = top.enter_context(tc.tile_pool(name="xp", bufs=1))
        wqp = top.enter_context(tc.tile_pool(name="wqp", bufs=1))
        kvp = top.enter_context(tc.tile_pool(name="kvp", bufs=1))
        qtp = top.enter_context(tc.tile_pool(name="qtp", bufs=3))
        otp = top.enter_context(tc.tile_pool(name="otp", bufs=1))
        esp = top.enter_context(tc.tile_pool(name="esp", bufs=8))
        wsp = top.enter_context(tc.tile_pool(name="wsp", bufs=6))
        wop = top.enter_context(tc.tile_pool(name="wop", bufs=5))
        ytp = top.enter_context(tc.tile_pool(name="ytp", bufs=2))
        msc = top.enter_context(tc.tile_pool(name="msc", bufs=1))
        rsp = top.enter_context(tc.tile_pool(name="rsp", bufs=3))
        lrp = top.enter_context(tc.tile_pool(name="lrp", bufs=3))
        pp = top.enter_context(tc.tile_pool(name="pp", bufs=8, space="PSUM"))

        # ---- input DMAs ----
        # sync: xkv then (inside A2) wv.  scalar: wk.  gpsimd: xq then wq.
        # First A1 matmul needs only xkv[0] (sync) + wk[0] (scalar), which
        # issue in parallel on different queues.
        xkv = [xp.tile([128, 768], BF16, tag=f"xkv{k}", name=f"xkv{k}")
               for k in range(16)]
        wkts = []
        for k in range(16):
            nc.sync.dma_start(xkv[k][:], xkvT[k * 128:(k + 1) * 128, :])
            wkt = wsp.tile([128, 512], BF16, tag="wk", name=f"wk{k}")
            nc.scalar.dma_start(wkt[:], wkv[k * 128:(k + 1) * 128, 0:512])
            wkts.append(wkt)
        xq = [xp.tile([128, 512], BF16, tag=f"xq{k}", name=f"xq{k}")
              for k in range(16)]
        for k in range(16):
            nc.gpsimd.dma_start(xq[k][:], xqT[k * 128:(k + 1) * 128, :])
        wqsb = [wqp.tile([128, 2048], BF16, tag=f"wq{k}", name=f"wq{k}")
                for k in range(16)]
        for k in range(16):
            nc.gpsimd.dma_start(wqsb[k][:], wq[k * 128:(k + 1) * 128, :])

        # ---- persistent K^T / V_aug tiles ----
        KT = [kvp.tile([64, 768], BF16, tag=f"kt{n}", name=f"kt{n}")
              for n in range(NKV)]
        VA = [kvp.tile([128, NKV * 65], BF16, tag=f"va{t}", name=f"va{t}")
              for t in range(6)]
        for t6 in range(6):
            oc = VA[t6].rearrange("q (h c) -> q h c", c=65)[:, :, 64:65]
            nc.vector.memset(oc, 1.0)

        # ---- A1: K^T [feat, t] ----
        kps = [pp.tile([128, 512], F32, tag="pb", name=f"kps{i}")
               for i in range(8)]
        for k in range(16):
            for f in range(4):
                for h in range(2):
                    nc.tensor.matmul(kps[f * 2 + h][:, 0:384],
                                     wkts[k][:, f * 128:(f + 1) * 128],
                                     xkv[k][:, h * 384:(h + 1) * 384],
                                     start=(k == 0), stop=(k == 15))
        for f in range(4):
            for h in range(2):
                ps = kps[f * 2 + h]
                nc.scalar.copy(KT[2 * f][0:64, h * 384:(h + 1) * 384],
                               ps[0:64, 0:384])
                nc.vector.tensor_copy(
                    KT[2 * f + 1][0:64, h * 384:(h + 1) * 384],
                    ps[64:128, 0:384])

        # ---- A2: V natural [t, feat] + ones ----
        vps = [pp.tile([128, 512], F32, tag="pb", name=f"vps{t}")
               for t in range(6)]
        for k in range(16):
            wvt = wsp.tile([128, 512], BF16, tag="wv", name=f"wv{k}")
            nc.sync.dma_start(wvt[:], wkv[k * 128:(k + 1) * 128, 512:1024])
            for t6 in range(6):
                nc.tensor.matmul(vps[t6][:],
                                 xkv[k][:, t6 * 128:(t6 + 1) * 128],
                                 wvt[:],
                                 start=(k == 0), stop=(k == 15))
        for t6 in range(6):
            dst = VA[t6].rearrange("q (h c) -> q h c", c=65)[:, :, 0:64]
            src = vps[t6].rearrange("q (h c) -> q h c", c=64)
            nc.vector.tensor_copy(dst, src)

        # ---- A3 + B software pipeline ----
        OT = [[otp.tile([128, 256], BF16, tag=f"ot{p}_{j}", name=f"ot{p}_{j}")
               for j in range(16)] for p in range(P)]
        QTt = {}
        qk_es = {}
        rsbs = {}

        def emit_a3(j):
            ps = pp.tile([128, 512], F32, tag="pb", name=f"qps{j}")
            for k in range(16):
                nc.tensor.matmul(ps[:],
                                 wqsb[k][:, j * 128:(j + 1) * 128],
                                 xq[k][:],
                                 start=(k == 0), stop=(k == 15))
            qt = qtp.tile([64, 1024], BF16, tag="qt", name=f"qt{j}")
            # qt col layout: p*512 + u*256 + s  (u = head within pair)
            dst = qt[0:64].rearrange("a (p u s) -> a p u s", p=2, u=2)
            nc.vector.tensor_copy(
                dst[:, :, 0, :], ps[0:64].rearrange("a (p s) -> a p s", p=2))
            nc.vector.tensor_copy(
                dst[:, :, 1, :], ps[64:128].rearrange("a (p s) -> a p s", p=2))
            QTt[j] = qt

        def emit_b_qk(j):
            n = j // 2
            for p in range(P):
                g = j * 2 + p
                for tt in range(4):
                    qk = pp.tile([128, 512], F32, tag="pb",
                                 name=f"qk{g}_{tt}")
                    tcol = p * 256 + tt * 128
                    nc.tensor.matmul(qk[:],
                                     KT[n][0:64, tcol:tcol + 128],
                                     QTt[j][0:64, p * 512:(p + 1) * 512],
                                     start=True, stop=True)
                    e = esp.tile([128, 512], BF16, tag="e", name=f"e{g}_{tt}")
                    nc.scalar.activation(e[:], qk[:], Exp, scale=float(SCALE))
                    qk_es[(g, tt)] = e

        def emit_b_pv(j):
            n = j // 2
            for p in range(P):
                g = j * 2 + p
                pv = pp.tile([128, 512], F32, tag="pb", name=f"pv{g}")
                for tt in range(4):
                    nc.tensor.matmul(pv[0:65, :],
                                     VA[p * 2 + tt][:, n * 65:(n + 1) * 65],
                                     qk_es.pop((g, tt))[:],
                                     start=(tt == 0), stop=(tt == 3))
                # 1/l = exp(-ln(l)) on ACT: both funcs live in the
                # natural_log_exp_and_others table, so no table reloads.
                # This keeps the softmax denominators per-group (no bursty
                # batched reciprocal chain blocking the DVE queue).
                lnl = lrp.tile([1, 512], F32, tag="lnl", name=f"lnl{g}")
                nc.scalar.activation(lnl[:], pv[64:65, 0:512], Ln)
                rr = lrp.tile([1, 512], BF16, tag="rr", name=f"rr{g}")
                with nc.allow_low_precision(reason="1/l in bf16"):
                    nc.scalar.activation(rr[:], lnl[:], Exp, scale=-1.0)
                rsbA = rsp.tile([128, 256], BF16, tag="rsbA", name=f"rsA{g}")
                nc.gpsimd.partition_broadcast(rsbA[:], rr[0:1, 0:256],
                                              channels=128)
                rsbB = rsp.tile([128, 256], BF16, tag="rsbB", name=f"rsB{g}")
                nc.gpsimd.partition_broadcast(rsbB[:], rr[0:1, 256:512],
                                              channels=128)
                # evacuate unnormalized immediately (psum drain must not
                # wait on the ln/exp/broadcast chain)
                nc.vector.tensor_copy(OT[p][j][0:64, :], pv[0:64, 0:256])
                nc.vector.tensor_copy(OT[p][j][64:128, :], pv[0:64, 256:512])
                rsbs[g] = (rsbA, rsbB)

        def emit_norm(j):
            # in-place normalize, emitted one iteration after the group's
            # PV so the ln->exp->broadcast chain (~4us) is already done
            # when the DVE reaches these ops (no queue head-blocking).
            for p in range(P):
                g = j * 2 + p
                rsbA, rsbB = rsbs.pop(g)
                nc.vector.tensor_tensor(OT[p][j][0:64, :], OT[p][j][0:64, :],
                                        rsbA[0:64, :], Mult)
                nc.vector.tensor_tensor(OT[p][j][64:128, :],
                                        OT[p][j][64:128, :],
                                        rsbB[64:128, :], Mult)

        for j in range(18):
            if 1 <= j <= 16:
                emit_b_qk(j - 1)
            if j < 16:
                emit_a3(j)
            if 1 <= j <= 16:
                emit_b_pv(j - 1)
            if j >= 2:
                emit_norm(j - 2)

        # ---- C: y = O @ wo  (nn pairs, 8 psum banks, [128,1024] wo) ----
        for half in range(2):
            acc = [pp.tile([128, 512], F32, tag="pb", name=f"acc{half}_{i}")
                   for i in range(8)]
            for k in range(16):
                wot = wop.tile([128, 1024], BF16, tag="wo",
                               name=f"wo{half}_{k}")
                nc.sync.dma_start(
                    wot[:],
                    wo[k * 128:(k + 1) * 128, half * 1024:(half + 1) * 1024])
                for n2 in range(2):
                    for p in range(P):
                        for m in range(2):
                            nc.tensor.matmul(
                                acc[n2 * 4 + p * 2 + m][:],
                                OT[p][k][:, m * 128:(m + 1) * 128],
                                wot[:, n2 * 512:(n2 + 1) * 512],
                                start=(k == 0), stop=(k == 15))
            for i, (p, m) in enumerate([(0, 0), (0, 1), (1, 0), (1, 1)]):
                yt = ytp.tile([128, 1024], F32, tag="yt",
                              name=f"yt{half}_{p}_{m}")
                for n2 in range(2):
                    eng = nc.vector if (i + n2) % 2 == 0 else nc.scalar
                    if eng is nc.vector:
                        eng.tensor_copy(yt[:, n2 * 512:(n2 + 1) * 512],
                                        acc[n2 * 4 + p * 2 + m][:])
                    else:
                        eng.copy(yt[:, n2 * 512:(n2 + 1) * 512],
                                 acc[n2 * 4 + p * 2 + m][:])
                r0 = p * 256 + m * 128
                q = (nc.sync, nc.gpsimd, nc.scalar, nc.sync)[i]
                q.dma_start(
                    y[r0:r0 + 128, half * 1024:(half + 1) * 1024], yt[:])

    nc.compile()
    return nc


def _get_nc():
    if "nc" not in _CACHE:
        _CACHE["nc"] = _build()
    return _CACHE["nc"]


def make_in_maps(x, wq, wkv, wo):
    x = np.asarray(x, dtype=np.float32)
    wq_b = np.asarray(wq, dtype=BFNP)
    wkv_b = np.asarray(wkv, dtype=BFNP)
    wo_b = np.asarray(wo, dtype=BFNP)
    in_maps = []
    for c in range(N_CORES):
        b, v0 = c // 4, V0S[c % 4]
        xq_c = np.ascontiguousarray(
            np.concatenate([x[b, v0].T, x[b, v0 + 2].T], axis=1)).astype(BFNP)
        xkv_c = np.ascontiguousarray(np.concatenate(
            [x[b, (v0 - 1) % V].T, x[b, (v0 + 1) % V].T,
             x[b, (v0 + 3) % V].T], axis=1)).astype(BFNP)
        in_maps.append({
            "xqT": xq_c, "xkvT": xkv_c,
            "wq": wq_b, "wkv": wkv_b, "wo": wo_b,
        })
    return in_maps


def kernel(x, wq, wkv, wo):
    from concourse.bass_utils import run_bass_kernel_spmd

    nc = _get_nc()
    in_maps = make_in_maps(x, wq, wkv, wo)
    res = run_bass_kernel_spmd(nc, in_maps, list(range(N_CORES)),
                               trace=False)
    out = np.empty((B, V, S, D), np.float32)
    for c in range(N_CORES):
        yc = res.results[c]["y"]
        b, v0 = c // 4, V0S[c % 4]
        out[b, v0] = yc[0:S]
        out[b, v0 + 2] = yc[S:2 * S]
    return out


# revision 30
# speedup vs baseline: 1.3819x; 1.3819x over previous
"""CrossViewAttention Trainium2 kernel (v7).

Shards the B*V=16 (batch, view) attention instances across 8 NeuronCores,
2 per core, paired as (b, v) and (b, v+2) so the two instances share KV
source view v+1 (each instance attends over views v-1, v+1 circular).
Per core the 3 distinct KV source views are projected once (25% fewer
K/V projection FLOPs). All matmul operands are bf16 (fp32 PSUM).

Schedule (single rotating 8-bank PSUM pool, no phase barriers):
  A1  K^T = wk^T @ x_kv^T   [feat, t]   (drain split ACT+DVE -> KT bf16)
  A2  V   = x_kv @ wv       [t, feat]   (+ ones col -> VA, DVE drain)
  A3+B software pipeline per head-pair j (lag 1):
      emit QK(j-1) -> emit A3(j) -> emit PV(j-1)
      so the ACT-engine exp of step j-1 hides under A3(j) matmuls.
  Softmax denominators l come free from PV's ones column. Per group the
  l row is staged at partition 0 (DVE), DMA-scattered into an
  [8-row x batch] quadrant-aligned stack, and each batch of 8 groups
  gets ONE [8,512] DVE reciprocal (HW reciprocal costs free-size*8cyc
  regardless of partition count). The 1/l rows are column-folded back
  to partition 0 by one DMA per batch; normalization (gpsimd
  partition_broadcast + in-place multiplies, all on the otherwise-idle
  Pool engine) is paced 2 groups per pipeline step so it never gates
  phase C. partition_broadcast HW quirk: dst/src partition base must
  be 0 (src column offsets are fine) - hence the fold to partition 0.
  C   y = O @ wo  (nn pairs, 8 psum banks, [128,1024] wo tiles; yt
      drains on DVE, y DMAs spread across queues)
"""
import numpy as np
import ml_dtypes

B, V, S, D = 2, 8, 256, 2048
NH, NKV, KVR = 32, 8, 2
HD = D // NH  # 64
G = NH // NKV  # 4
N_CORES = 8
P = 2  # instances per core
SCALE = 1.0 / np.sqrt(HD)
BFNP = ml_dtypes.bfloat16
V0S = (0, 1, 4, 5)  # per-core first view; pair is (v0, v0+2)

_CACHE = {}


def _build():
    import concourse.tile as tile
    import concourse.mybir as mybir
    from concourse import bacc
    from contextlib import ExitStack

    F32 = mybir.dt.float32
    BF16 = mybir.dt.bfloat16
    Exp = mybir.ActivationFunctionType.Exp
    Ln = mybir.ActivationFunctionType.Ln
    Mult = mybir.AluOpType.mult

    nc = bacc.Bacc("TRN2", target_bir_lowering=False, debug=False,
                   num_devices=N_CORES)
    xqT = nc.dram_tensor("xqT", [D, P * S], BF16, kind="ExternalInput").ap()
    xkvT = nc.dram_tensor("xkvT", [D, 768], BF16, kind="ExternalInput").ap()
    wq = nc.dram_tensor("wq", [D, D], BF16, kind="ExternalInput").ap()
    wkv = nc.dram_tensor("wkv", [D, 1024], BF16, kind="ExternalInput").ap()
    wo = nc.dram_tensor("wo", [D, D], BF16, kind="ExternalInput").ap()
    y = nc.dram_tensor("y", [P * S, D], F32, kind="ExternalOutput").ap()

    with tile.TileContext(nc) as tc, ExitStack() as top:
        xp = top.enter_context(tc.tile_pool(name="xp", bufs=1))
        wqp = top.enter_context(tc.tile_pool(name="wqp", bufs=1))
        kvp = top.enter_context(tc.tile_pool(name="kvp", bufs=1))
        qtp = top.enter_context(tc.tile_pool(name="qtp", bufs=3))
        otp = top.enter_context(tc.tile_pool(name="otp", bufs=1))
        esp = top.enter_context(tc.tile_pool(name="esp", bufs=8))
        wsp = top.enter_context(tc.tile_pool(name="wsp", bufs=6))
        wop = top.enter_context(tc.tile_pool(name="wop", bufs=5))
        ytp = top.enter_context(tc.tile_pool(name="ytp", bufs=2))
        msc = top.enter_context(tc.tile_pool(name="msc", bufs=1))
        rsp = top.enter_context(tc.tile_pool(name="rsp", bufs=4))
        lrp = top.enter_context(tc.tile_pool(name="lrp", bufs=4))
        pp = top.enter_context(tc.tile_pool(name="pp", bufs=8, space="PSUM"))

        # ---- input DMAs ----
        # sync: xkv then (inside A2) wv.  scalar: wk.  gpsimd: xq then wq.
        # First A1 matmul needs only xkv[0] (sync) + wk[0] (scalar), which
        # issue in parallel on different queues.
        xkv = [xp.tile([128, 768], BF16, tag=f"xkv{k}", name=f"xkv{k}")
               for k in range(16)]
        wkts = []
        for k in range(16):
            nc.sync.dma_start(xkv[k][:], xkvT[k * 128:(k + 1) * 128, :])
            wkt = wsp.tile([128, 512], BF16, tag="wk", name=f"wk{k}")
            nc.scalar.dma_start(wkt[:], wkv[k * 128:(k + 1) * 128, 0:512])
            wkts.append(wkt)
        xq = [xp.tile([128, 512], BF16, tag=f"xq{k}", name=f"xq{k}")
              for k in range(16)]
        for k in range(16):
            nc.gpsimd.dma_start(xq[k][:], xqT[k * 128:(k + 1) * 128, :])
        wqsb = [wqp.tile([128, 2048], BF16, tag=f"wq{k}", name=f"wq{k}")
                for k in range(16)]
        for k in range(16):
            nc.gpsimd.dma_start(wqsb[k][:], wq[k * 128:(k + 1) * 128, :])

        # ---- persistent K^T / V_aug tiles ----
        KT = [kvp.tile([64, 768], BF16, tag=f"kt{n}", name=f"kt{n}")
              for n in range(NKV)]
        VA = [kvp.tile([128, NKV * 65], BF16, tag=f"va{t}", name=f"va{t}")
              for t in range(6)]
        for t6 in range(6):
            oc = VA[t6].rearrange("q (h c) -> q h c", c=65)[:, :, 64:65]
            nc.vector.memset(oc, 1.0)

        # ---- A1: K^T [feat, t] ----
        kps = [pp.tile([128, 512], F32, tag="pb", name=f"kps{i}")
               for i in range(8)]
        for k in range(16):
            for f in range(4):
                for h in range(2):
                    nc.tensor.matmul(kps[f * 2 + h][:, 0:384],
                                     wkts[k][:, f * 128:(f + 1) * 128],
                                     xkv[k][:, h * 384:(h + 1) * 384],
                                     start=(k == 0), stop=(k == 15))
        for f in range(4):
            for h in range(2):
                ps = kps[f * 2 + h]
                nc.scalar.copy(KT[2 * f][0:64, h * 384:(h + 1) * 384],
                               ps[0:64, 0:384])
                nc.vector.tensor_copy(
                    KT[2 * f + 1][0:64, h * 384:(h + 1) * 384],
                    ps[64:128, 0:384])

        # ---- A2: V natural [t, feat] + ones ----
        vps = [pp.tile([128, 512], F32, tag="pb", name=f"vps{t}")
               for t in range(6)]
        for k in range(16):
            wvt = wsp.tile([128, 512], BF16, tag="wv", name=f"wv{k}")
            nc.sync.dma_start(wvt[:], wkv[k * 128:(k + 1) * 128, 512:1024])
            for t6 in range(6):
                nc.tensor.matmul(vps[t6][:],
                                 xkv[k][:, t6 * 128:(t6 + 1) * 128],
                                 wvt[:],
                                 start=(k == 0), stop=(k == 15))
        for t6 in range(6):
            dst = VA[t6].rearrange("q (h c) -> q h c", c=65)[:, :, 0:64]
            src = vps[t6].rearrange("q (h c) -> q h c", c=64)
            nc.vector.tensor_copy(dst, src)

        # ---- A3 + B software pipeline ----
        OT = [[otp.tile([128, 256], BF16, tag=f"ot{p}_{j}", name=f"ot{p}_{j}")
               for j in range(16)] for p in range(P)]
        QTt = {}
        qk_es = {}
        rsbs = {}
        lrows = {}

        def emit_a3(j):
            ps = pp.tile([128, 512], F32, tag="pb", name=f"qps{j}")
            for k in range(16):
                nc.tensor.matmul(ps[:],
                                 wqsb[k][:, j * 128:(j + 1) * 128],
                                 xq[k][:],
                                 start=(k == 0), stop=(k == 15))
            qt = qtp.tile([64, 1024], BF16, tag="qt", name=f"qt{j}")
            # qt col layout: p*512 + u*256 + s  (u = head within pair)
            dst = qt[0:64].rearrange("a (p u s) -> a p u s", p=2, u=2)
            nc.vector.tensor_copy(
                dst[:, :, 0, :], ps[0:64].rearrange("a (p s) -> a p s", p=2))
            nc.vector.tensor_copy(
                dst[:, :, 1, :], ps[64:128].rearrange("a (p s) -> a p s", p=2))
            QTt[j] = qt

        def emit_b_qk(j):
            n = j // 2
            for p in range(P):
                g = j * 2 + p
                for tt in range(4):
                    qk = pp.tile([128, 512], F32, tag="pb",
                                 name=f"qk{g}_{tt}")
                    tcol = p * 256 + tt * 128
                    nc.tensor.matmul(qk[:],
                                     KT[n][0:64, tcol:tcol + 128],
                                     QTt[j][0:64, p * 512:(p + 1) * 512],
                                     start=True, stop=True)
                    e = esp.tile([128, 512], BF16, tag="e", name=f"e{g}_{tt}")
                    nc.scalar.activation(e[:], qk[:], Exp, scale=float(SCALE))
                    qk_es[(g, tt)] = e

        def emit_b_pv(j):
            n = j // 2
            for p in range(P):
                g = j * 2 + p
                pv = pp.tile([128, 512], F32, tag="pb", name=f"pv{g}")
                for tt in range(4):
                    nc.tensor.matmul(pv[0:65, :],
                                     VA[p * 2 + tt][:, n * 65:(n + 1) * 65],
                                     qk_es.pop((g, tt))[:],
                                     start=(tt == 0), stop=(tt == 3))
                # stage l at partition 0 and evacuate O^T unnormalized:
                # the psum drain must not wait on the 1/l chain.
                lrow = lrp.tile([1, 512], BF16, tag="lr", name=f"lr{g}")
                with nc.allow_low_precision(reason="l sums in bf16"):
                    nc.vector.tensor_copy(lrow[:], pv[64:65, 0:512])
                lrows[g] = lrow
                nc.vector.tensor_copy(OT[p][j][0:64, :], pv[0:64, 0:256])
                nc.vector.tensor_copy(OT[p][j][64:128, :], pv[0:64, 256:512])

        def emit_lninv(j):
            # lag 1: 1/l = exp(-ln(l)) on ACT (both funcs share the
            # natural_log_exp_and_others table -> no table reloads), then
            # Pool broadcasts. Lagging keeps ln off the latency-critical
            # part of the ACT queue (the exps feeding PV).
            for p in range(P):
                g = j * 2 + p
                lrow = lrows.pop(g)
                lnl = lrp.tile([1, 512], F32, tag="lnl", name=f"lnl{g}")
                nc.scalar.activation(lnl[:], lrow[:], Ln)
                rr = lrp.tile([1, 512], BF16, tag="rr", name=f"rr{g}")
                with nc.allow_low_precision(reason="1/l in bf16"):
                    nc.scalar.activation(rr[:], lnl[:], Exp, scale=-1.0)
                rsbA = rsp.tile([128, 256], BF16, tag="rsbA", name=f"rsA{g}")
                nc.gpsimd.partition_broadcast(rsbA[:], rr[0:1, 0:256],
                                              channels=128)
                rsbB = rsp.tile([128, 256], BF16, tag="rsbB", name=f"rsB{g}")
                nc.gpsimd.partition_broadcast(rsbB[:], rr[0:1, 256:512],
                                              channels=128)
                rsbs[g] = (rsbA, rsbB)

        def emit_norm(j):
            # lag 2: in-place normalize; the whole 1/l chain is done by
            # now so these never head-block the DVE queue.
            for p in range(P):
                g = j * 2 + p
                rsbA, rsbB = rsbs.pop(g)
                nc.vector.tensor_tensor(OT[p][j][0:64, :], OT[p][j][0:64, :],
                                        rsbA[0:64, :], Mult)
                nc.vector.tensor_tensor(OT[p][j][64:128, :],
                                        OT[p][j][64:128, :],
                                        rsbB[64:128, :], Mult)

        for j in range(19):
            if 1 <= j <= 16:
                emit_b_qk(j - 1)
            if j < 16:
                emit_a3(j)
            if 1 <= j <= 16:
                emit_b_pv(j - 1)
            if 2 <= j <= 17:
                emit_lninv(j - 2)
            if j >= 3:
                emit_norm(j - 3)

        # ---- C: y = O @ wo  (nn pairs, 8 psum banks, [128,1024] wo) ----
        for half in range(2):
            acc = [pp.tile([128, 512], F32, tag="pb", name=f"acc{half}_{i}")
                   for i in range(8)]
            for k in range(16):
                wot = wop.tile([128, 1024], BF16, tag="wo",
                               name=f"wo{half}_{k}")
                nc.sync.dma_start(
                    wot[:],
                    wo[k * 128:(k + 1) * 128, half * 1024:(half + 1) * 1024])
                for n2 in range(2):
                    for p in range(P):
                        for m in range(2):
                            nc.tensor.matmul(
                                acc[n2 * 4 + p * 2 + m][:],
                                OT[p][k][:, m * 128:(m + 1) * 128],
                                wot[:, n2 * 512:(n2 + 1) * 512],
                                start=(k == 0), stop=(k == 15))
            for i, (p, m) in enumerate([(0, 0), (0, 1), (1, 0), (1, 1)]):
                yt = ytp.tile([128, 1024], F32, tag="yt",
                              name=f"yt{half}_{p}_{m}")
                for n2 in range(2):
                    eng = nc.vector if (i + n2) % 2 == 0 else nc.scalar
                    if eng is nc.vector:
                        eng.tensor_copy(yt[:, n2 * 512:(n2 + 1) * 512],
                                        acc[n2 * 4 + p * 2 + m][:])
                    else:
                        eng.copy(yt[:, n2 * 512:(n2 + 1) * 512],
                                 acc[n2 * 4 + p * 2 + m][:])
                r0 = p * 256 + m * 128
                q = (nc.sync, nc.gpsimd, nc.scalar, nc.sync)[i]
                q.dma_start(
                    y[r0:r0 + 128, half * 1024:(half + 1) * 1024], yt[:])

    nc.compile()
    return nc


def _get_nc():
    if "nc" not in _CACHE:
        _CACHE["nc"] = _build()
    return _CACHE["nc"]


def make_in_maps(x, wq, wkv, wo):
    x = np.asarray(x, dtype=np.float32)
    wq_b = np.asarray(wq, dtype=BFNP)
    wkv_b = np.asarray(wkv, dtype=BFNP)
    wo_b = np.asarray(wo, dtype=BFNP)
    in_maps = []
    for c in range(N_CORES):
        b, v0 = c // 4, V0S[c % 4]
        xq_c = np.ascontiguousarray(
            np.concatenate([x[b, v0].T, x[b, v0 + 2].T], axis=1)).astype(BFNP)
        xkv_c = np.ascontiguousarray(np.concatenate(
            [x[b, (v0 - 1) % V].T, x[b, (v0 + 1) % V].T,
             x[b, (v0 + 3) % V].T], axis=1)).astype(BFNP)
        in_maps.append({
            "xqT": xq_c, "xkvT": xkv_c,
            "wq": wq_b, "wkv": wkv_b, "wo": wo_b,
        })
    return in_maps


def kernel(x, wq, wkv, wo):
    from concourse.bass_utils import run_bass_kernel_spmd

    nc = _get_nc()
    in_maps = make_in_maps(x, wq, wkv, wo)
    res = run_bass_kernel_spmd(nc, in_maps, list(range(N_CORES)),
                               trace=False)
    out = np.empty((B, V, S, D), np.float32)
    for c in range(N_CORES):
        yc = res.results[c]["y"]
        b, v0 = c // 4, V0S[c % 4]
        out[b, v0] = yc[0:S]
        out[b, v0 + 2] = yc[S:2 * S]
    return out
